# revision 61
# baseline (speedup 1.0000x reference)
"""Trainium2 Bass kernel for nn_BaichuanAttention_4801773437527.

Sequence-sharded across 8 NeuronCores: core c handles 512 query rows
(batch c//4, seq block (c%4)*512). Host pre-transposes hidden and
pre-packs bf16 weights into tile-contiguous layouts. Everything stays
resident in SBUF (no DRAM scratch round-trips). All heavy matmuls run
in bf16 on the PE array with fp32 PSUM accumulation.

Phases per core:
  B: kv projection (16 f-tiles x 1024 tokens) + smooth + rope(k) +
     transpose(v), pipelined.
  C: q projection (32 heads x 512 own tokens) + rope.
  E: windowed attention per head (QK, exp on ACT, 0/1 mask mul on DVE
     in bf16, ones-matmul softmax sum, PV, normalize).
  F: o_proj with streamed bf16 weights.
Output is row-sharded so there are no collectives.

_build_program(repeat=N) unrolls the whole computation N times inside
one NEFF -- used for wall-clock HW benchmarking (per-rep time =
(wall[R] - wall[1]) / (R - 1), staging/dispatch overheads cancel).
"""
import sys
sys.path.insert(0, '/opt/trn_rl_repo')
from contextlib import ExitStack
import numpy as np

B, S, HID = 2, 2048, 4096
H, KV, D = 32, 8, 128
WINDOW = 512
CHUNK = 512
NCORES = 8
ROPE_THETA = 10000.0
F = (H + 2 * KV) * D          # 6144
NFT = F // 128                # 48 f-tiles (0..31 q, 32..39 k, 40..47 v)
KT = HID // 128               # 32 contraction tiles
SCALE = float(D) ** -0.5

_PROGRAMS = {}
TRACE = False
REPEAT = 1
_LAST_RESULTS = None


def _apply_patches():
    """This walrus build allows 1 sync wait per instruction (2 for
    EventSemaphore). Spill extra waits onto same-engine no-ops."""
    import concourse.mybir as mybir
    import concourse.tile as tile
    from concourse.vector_clock import ScopedClock

    if getattr(tile.TileContext, "_wait_patch_applied", False):
        return

    orig_lower = tile.TileContext._lower_ordered_insts
    counter = [0]

    def spill(ordered):
        for insts in ordered.values():
            new_insts = []
            for inst in insts:
                si = getattr(inst, "sync_info", None)
                if si is not None and type(inst).__name__.startswith("Inst"):
                    waits = list(si.on_wait)
                    cap = 2 if isinstance(inst, mybir.InstEventSemaphore) else 1
                    if len(waits) > cap:
                        for w in waits[cap:]:
                            counter[0] += 1
                            new_insts.append(mybir.InstNoOp(
                                name=f"wspill-{counter[0]}",
                                sync_info=mybir.SyncInfo(on_wait=[w], on_update=[]),
                                bass_nofuse=True,
                                engine=inst.engine,
                            ))
                        inst.sync_info = mybir.SyncInfo(
                            on_wait=waits[:cap], on_update=list(si.on_update))
                new_insts.append(inst)
            insts[:] = new_insts

    def patched_lower(self, ordered):
        spill(ordered)
        return orig_lower(self, ordered)

    def patched_drain_and_barrier(self, tick_clock, wait_clock):
        nc = self.nc
        collector = nc.sync.nop(nofuse=True)
        wait_clock.add_sem_waits(
            collector.ins, ScopedClock({None: tick_clock.global_clock}))
        si = collector.ins.sync_info
        waits = list(si.on_wait) if si is not None else []
        if len(waits) > 1:
            collector.ins.sync_info = mybir.SyncInfo(
                on_wait=[waits[0]], on_update=list(si.on_update))
            for w in waits[1:]:
                n = nc.sync.nop(nofuse=True)
                n.ins.sync_info = mybir.SyncInfo(on_wait=[w], on_update=[])
        nc.sync.drain()
        nc.all_engine_barrier()
        assert self.sems is not None
        popped = nc._tile_sem_poison_stack.pop()
        assert popped is self._sem_poison
        nc.clear_and_free_semaphores(list(self.sems.allocated().values()))
        nc.all_engine_barrier()

    tile.TileContext._lower_ordered_insts = patched_lower
    tile.TileContext._drain_and_barrier = patched_drain_and_barrier
    tile.TileContext._wait_patch_applied = True


def _build_program(repeat=1, upto="F"):
    import concourse.bass as bass
    import concourse.mybir as mybir
    import concourse.tile as tile
    from concourse.masks import make_identity

    _apply_patches()

    f32 = mybir.dt.float32
    f32r = mybir.dt.float32r
    bf16 = mybir.dt.bfloat16
    MUL = mybir.AluOpType.mult
    ADD = mybir.AluOpType.add
    EXP = mybir.ActivationFunctionType.Exp

    nc = bass.Bass()
    # [p, kt, t]: hidT[p, kt, t] = hidden[token t, kt*128 + p], bf16
    hidT = nc.dram_tensor("hidT", [128, KT, 2 * CHUNK], bf16,
                          kind="ExternalInput")
    # [ft, p, kt, f]: w_pack tile-packed bf16
    wpk = nc.dram_tensor("wpk", [NFT, 128, KT, 128], bf16,
                         kind="ExternalInput")
    # [hc, p, kt, h]: w_o tile-packed bf16, 8 chunks of 512 hid cols
    wob = nc.dram_tensor("wob", [8, 128, KT, 512], bf16,
                         kind="ExternalInput")
    costab = nc.dram_tensor("costab", [128, 1024], bf16, kind="ExternalInput")
    sintab = nc.dram_tensor("sintab", [128, 1024], bf16, kind="ExternalInput")
    # multiplicative 0/1 mask, flat [p, tt*512 + q]
    maskmul = nc.dram_tensor("maskmul", [128, 4096], bf16,
                             kind="ExternalInput")
    filt = nc.dram_tensor("filt", [128, 4 * KV], f32, kind="ExternalInput")
    rotm = nc.dram_tensor("rotm", [128, 128], bf16, kind="ExternalInput")
    out = nc.dram_tensor("out", [CHUNK, HID], f32, kind="ExternalOutput")

    out_r = out[:].rearrange("(t p) h -> t p h", p=128)         # [4,128,4096]

    with tile.TileContext(nc) as tc, ExitStack() as top:
        constp = top.enter_context(tc.tile_pool(name="const", bufs=1))
        ident_f = constp.tile([128, 128], f32, tag="identf")
        make_identity(nc, ident_f[:])
        ident_bf = constp.tile([128, 128], bf16, tag="identbf")
        nc.vector.tensor_copy(ident_bf[:], ident_f[:])
        filt_bc = constp.tile([128, 4 * KV], f32, tag="filtbc")
        nc.sync.dma_start(filt_bc[:], filt[:])
        ones_f = constp.tile([128, 1], f32, tag="onesf")
        nc.gpsimd.memset(ones_f[:], 1.0)
        ones_bf = constp.tile([128, 1], bf16, tag="onesbf")
        nc.vector.tensor_copy(ones_bf[:], ones_f[:])
        ones2d_f = constp.tile([128, 128], f32, tag="ones2df")
        nc.gpsimd.memset(ones2d_f[:], 1.0)
        onesrow_r = constp.tile([1, 128], f32r, tag="onesrowr")
        nc.vector.tensor_copy(onesrow_r[:], ones2d_f[0:1, :])
        rot_sb = constp.tile([128, 128], bf16, tag="rotsb")
        nc.sync.dma_start(rot_sb[:], rotm[:])
        cos_sb = constp.tile([128, 1024], bf16, tag="cos")
        sin_sb = constp.tile([128, 1024], bf16, tag="sin")
        nc.sync.dma_start(cos_sb[:], costab[:])
        nc.sync.dma_start(sin_sb[:], sintab[:])
        for _rep in range(repeat):
            _one_rep(nc, tc, bass, mybir, tile,
                     hidT, wpk, wob, maskmul, out_r,
                     ident_bf, filt_bc, ones_bf, onesrow_r, rot_sb,
                     cos_sb, sin_sb, upto)
    return nc


def _one_rep(nc, tc, bass, mybir, tile, hidT, wpk, wob, maskmul, out_r,
             ident_bf, filt_bc, ones_bf, onesrow_r, rot_sb, cos_sb, sin_sb,
             upto="F"):
    f32 = mybir.dt.float32
    f32r = mybir.dt.float32r
    bf16 = mybir.dt.bfloat16
    MUL = mybir.AluOpType.mult
    ADD = mybir.AluOpType.add
    EXP = mybir.ActivationFunctionType.Exp

    # attention output, lives until o_proj (opened early for LIFO order)
    es_at = ExitStack()
    atp = es_at.enter_context(tc.tile_pool(name="atp", bufs=1))
    attnT = atp.tile([128, H, 512], bf16, tag="attnT")
    # persistent activations (q/k/v), live until end of attention
    es_act1 = ExitStack()
    actp = es_act1.enter_context(tc.tile_pool(name="actp", bufs=1))
    qfin = actp.tile([128, H * 512], bf16, tag="qfin")
    kfin = actp.tile([128, KV * 1024], bf16, tag="kfin")
    vT = actp.tile([128, KV * 8 * 128], bf16, tag="vT")

    # ---- phase A: load hidT (already transposed on host) ----
    es_hT = ExitStack()
    hTp = es_hT.enter_context(tc.tile_pool(name="hTp", bufs=1))
    hT = hTp.tile([128, KT, 1024], bf16, tag="hT")
    for kc in range(8):
        nc.sync.dma_start(hT[:, 4 * kc:4 * kc + 4, :],
                          hidT[:, 4 * kc:4 * kc + 4, :])

    es_wp = ExitStack()
    wpB = es_wp.enter_context(tc.tile_pool(name="wpB", bufs=2))
    es_rot = ExitStack()
    rotp = es_rot.enter_context(
        tc.tile_pool(name="rotp", bufs=1, space="PSUM"))

    # ---- phase B: kv projection + smooth + rope(k) / transpose(v) ----
    with tc.tile_pool(name="kvmm", bufs=2, space="PSUM") as kvmm, \
         tc.tile_pool(name="vtp", bufs=2, space="PSUM") as vtp, \
         tc.tile_pool(name="smpA", bufs=1) as smpA, \
         tc.tile_pool(name="smpB", bufs=2) as smpB:
        for ft in range(H, NFT):          # 32..39 k tiles, 40..47 v tiles
            kind = 0 if ft < H + KV else 1
            hkv = ft - H - kind * KV
            wt = wpB.tile([128, KT, 128], bf16, tag="wt")
            nc.sync.dma_start(wt[:], wpk[ft])
            ps = kvmm.tile([128, 1024], f32, tag="kvps")
            for half in (0, 1):
                for kt in range(KT):
                    nc.tensor.matmul(
                        ps[:, half * 512:(half + 1) * 512],
                        wt[:, kt, :],
                        hT[:, kt, half * 512:(half + 1) * 512],
                        start=(kt == 0), stop=(kt == KT - 1))
            # smooth: sm[t] = f1*ps[t] + f0*ps[t-1]   (ps in PSUM)
            fidx = 2 * kind * KV + hkv
            tmp = smpA.tile([128, 1024], f32, tag="smtmp")
            nc.vector.tensor_scalar_mul(
                tmp[:], ps[:], filt_bc[:, fidx + KV:fidx + KV + 1])
            sm = smpA.tile([128, 1024], f32, tag="smout")
            nc.vector.tensor_copy(sm[:, 0:1], tmp[:, 0:1])
            nc.vector.scalar_tensor_tensor(
                sm[:, 1:1024], ps[:, 0:1023],
                filt_bc[:, fidx:fidx + 1], tmp[:, 1:1024], MUL, ADD)
            if kind == 0:
                # k: rope then store bf16
                kbf = smpB.tile([128, 1024], bf16, tag="kvbf")
                nc.scalar.copy(kbf[:], sm[:])
                zk = rotp.tile([128, 1024], f32, tag="rotps")
                nc.tensor.matmul(zk[:, 0:512], rot_sb[:], kbf[:, 0:512],
                                 start=True, stop=True)
                nc.tensor.matmul(zk[:, 512:1024], rot_sb[:],
                                 kbf[:, 512:1024], start=True, stop=True)
                t1 = smpA.tile([128, 1024], bf16, tag="kt1")
                t2 = smpA.tile([128, 1024], bf16, tag="kt2")
                nc.vector.tensor_tensor(t1[:], kbf[:], cos_sb[:], MUL)
                nc.vector.tensor_tensor(t2[:], zk[:], sin_sb[:], MUL)
                nc.vector.tensor_tensor(
                    kfin[:, hkv * 1024:(hkv + 1) * 1024],
                    t1[:], t2[:], ADD)
            else:
                # v: convert to bf16, transpose into [t, d] tiles
                vbf = smpB.tile([128, 1024], bf16, tag="kvbf")
                nc.scalar.copy(vbf[:], sm[:])
                for half in (0, 1):
                    pv = vtp.tile([128, 512], bf16, tag="vtps")
                    for tt in range(4):
                        nc.tensor.transpose(
                            pv[:, tt * 128:(tt + 1) * 128],
                            vbf[:, half * 512 + tt * 128:
                                half * 512 + (tt + 1) * 128],
                            ident_bf[:])
                    base = (hkv * 8 + half * 4) * 128
                    nc.scalar.copy(vT[:, base:base + 512], pv[:])

    if upto == "B":
        es_rot.close(); es_wp.close(); es_hT.close()
        es_act1.close(); es_at.close()
        return

    # ---- phase C: q projection + rope (own half only) ----
    with tc.tile_pool(name="qmm", bufs=2, space="PSUM") as qmm, \
         tc.tile_pool(name="qev", bufs=3) as qev:
        for h in range(H):
            wt = wpB.tile([128, KT, 128], bf16, tag="wt")
            nc.sync.dma_start(wt[:], wpk[h])
            ps = qmm.tile([128, 512], f32, tag="qps")
            for kt in range(KT):
                nc.tensor.matmul(
                    ps[:], wt[:, kt, :], hT[:, kt, 512:1024],
                    start=(kt == 0), stop=(kt == KT - 1))
            qsb = qev.tile([128, 512], bf16, tag="qsb")
            nc.scalar.copy(qsb[:], ps[:])
            zps = rotp.tile([128, 1024], f32, tag="rotps")
            nc.tensor.matmul(zps[:, 0:512], rot_sb[:], qsb[:],
                             start=True, stop=True)
            t1 = qev.tile([128, 512], bf16, tag="qt1")
            t2 = qev.tile([128, 512], bf16, tag="qt2")
            nc.vector.tensor_tensor(
                t1[:], qsb[:], cos_sb[:, 512:1024], MUL)
            nc.vector.tensor_tensor(
                t2[:], zps[:, 0:512], sin_sb[:, 512:1024], MUL)
            nc.vector.tensor_tensor(
                qfin[:, h * 512:(h + 1) * 512], t1[:], t2[:], ADD)
    es_rot.close()
    es_wp.close()
    es_hT.close()

    if upto == "C":
        es_act1.close(); es_at.close()
        return

    # ---- phase E: attention ----
    es_wop = ExitStack()
    wop = es_wop.enter_context(tc.tile_pool(name="wop", bufs=2))
    with tc.tile_pool(name="mskp", bufs=1) as mskp, \
         tc.tile_pool(name="scp", bufs=2, space="PSUM") as scp, \
         tc.tile_pool(name="pvp", bufs=2, space="PSUM") as pvp, \
         tc.tile_pool(name="smps", bufs=1, space="PSUM") as smps, \
         tc.tile_pool(name="rbp", bufs=1, space="PSUM") as rbp, \
         tc.tile_pool(name="prp", bufs=2) as prp, \
         tc.tile_pool(name="mscp", bufs=2) as mscp:
        masks_sb = mskp.tile([128, 4096], bf16, tag="masks")
        nc.sync.dma_start(masks_sb[:], maskmul[:])
        for h in range(H):
            g = h // (H // KV)
            qv = qfin[:, h * 512:(h + 1) * 512]
            probsT = prp.tile([128, 4096], bf16, tag="probsT")
            for pair in range(4):
                sps = scp.tile([128, 1024], f32, tag="sc")
                for j in (0, 1):
                    tt = 2 * pair + j
                    nc.tensor.matmul(
                        sps[:, j * 512:(j + 1) * 512],
                        kfin[:, g * 1024 + tt * 128:
                             g * 1024 + (tt + 1) * 128],
                        qv, start=True, stop=True)
                # exp(scale*s) from PSUM, then 0/1 mask multiply (bf16)
                pview = probsT[:, pair * 1024:(pair + 1) * 1024]
                nc.scalar.activation(pview, sps[:], EXP, scale=SCALE)
                nc.vector.tensor_tensor(
                    pview, pview,
                    masks_sb[:, pair * 1024:(pair + 1) * 1024], MUL)
            sumps = smps.tile([1, 512], f32, tag="sum")
            for tt in range(8):
                nc.tensor.matmul(
                    sumps[:], ones_bf[:],
                    probsT[:, tt * 512:(tt + 1) * 512],
                    start=(tt == 0), stop=(tt == 7))
            rec = mscp.tile([1, 512], f32r, tag="rec")
            with nc.allow_low_precision(reason="f32r recip for bcast mm"):
                nc.vector.reciprocal(rec[:], sumps[:])
            recb_ps = rbp.tile([128, 512], f32, tag="recbps")
            nc.tensor.matmul(recb_ps[:], onesrow_r[:], rec[:],
                             start=True, stop=True)
            recb = mscp.tile([128, 512], f32, tag="recb")
            nc.scalar.copy(recb[:], recb_ps[:])
            pvs = pvp.tile([128, 512], f32, tag="pv")
            for tt in range(8):
                nc.tensor.matmul(
                    pvs[:], vT[:, (g * 8 + tt) * 128:(g * 8 + tt + 1) * 128],
                    probsT[:, tt * 512:(tt + 1) * 512],
                    start=(tt == 0), stop=(tt == 7))
            nc.vector.tensor_tensor(attnT[:, h, :], pvs[:], recb[:], MUL)

    if upto == "E":
        es_wop.close(); es_act1.close(); es_at.close()
        return

    # ---- phase F: o_proj ----
    with tc.tile_pool(name="opp", bufs=4, space="PSUM") as opp, \
         tc.tile_pool(name="oev", bufs=3) as oevp:
        for hc in range(8):
            wt = wop.tile([128, KT, 512], bf16, tag="wo")
            nc.sync.dma_start(wt[:], wob[hc])
            for st in range(4):
                ps = opp.tile([128, 512], f32, tag="ops")
                for ft in range(KT):
                    nc.tensor.matmul(
                        ps[:],
                        attnT[:, ft, st * 128:(st + 1) * 128],
                        wt[:, ft, :],
                        start=(ft == 0), stop=(ft == KT - 1))
                ev = oevp.tile([128, 512], f32, tag="oev")
                nc.scalar.copy(ev[:], ps[:])
                nc.sync.dma_start(
                    out_r[st, :, hc * 512:(hc + 1) * 512], ev[:])
    es_wop.close()
    es_act1.close()
    es_at.close()


def _host_tables(positions_b, s0):
    """cos/sin rope tables [128,1024] and 0/1 bf16 mask flat [128,4096]."""
    import ml_dtypes
    if s0 > 0:
        pos_prev = positions_b[s0 - 512:s0].astype(np.float64)
    else:
        pos_prev = np.zeros(512, np.float64)
    pos_own = positions_b[s0:s0 + 512].astype(np.float64)
    tpos = np.concatenate([pos_prev, pos_own])                   # [1024]
    inv = 1.0 / (ROPE_THETA ** (np.arange(64, dtype=np.float64) / 64.0))
    ang = inv[:, None] * tpos[None, :]                           # [64,1024]
    cos = np.cos(ang)
    sin = np.sin(ang)
    costab = np.concatenate([cos, cos], axis=0).astype(ml_dtypes.bfloat16)
    sintab = np.concatenate([sin, sin], axis=0).astype(ml_dtypes.bfloat16)

    t_idx = s0 - 512 + np.arange(1024)
    q_idx = s0 + np.arange(512)
    diff = q_idx[None, :] - t_idx[:, None]                       # [1024,512]
    valid = (diff >= 0) & (diff < WINDOW) & (t_idx[:, None] >= 0)
    maskarr = valid.reshape(8, 128, 512).transpose(1, 0, 2).reshape(128, 4096)
    return costab, sintab, maskarr.astype(ml_dtypes.bfloat16)


def _rot_matrix():
    import ml_dtypes
    R = np.zeros((128, 128), np.float32)
    for d in range(64):
        R[d + 64, d] = -1.0
        R[d, d + 64] = 1.0
    return R.astype(ml_dtypes.bfloat16)


def _prepare_in_maps(inputs):
    import ml_dtypes

    bf = ml_dtypes.bfloat16
    hidden = np.asarray(inputs["hidden_states"], dtype=np.float32)
    positions = np.asarray(inputs["positions"], dtype=np.int32)
    w_pack = np.asarray(inputs["w_pack"], dtype=np.float32)
    w_o = np.asarray(inputs["w_o"], dtype=np.float32)
    conv_k = np.asarray(inputs["conv_k"], dtype=np.float32)
    conv_v = np.asarray(inputs["conv_v"], dtype=np.float32)

    # tile-packed bf16 weights (shared across cores)
    wpk = np.ascontiguousarray(
        w_pack.reshape(KT, 128, NFT, 128).transpose(2, 1, 0, 3)).astype(bf)
    wob = np.ascontiguousarray(
        w_o.reshape(KT, 128, 8, 512).transpose(2, 1, 0, 3)).astype(bf)
    filt_arr = np.concatenate(
        [conv_k[0], conv_k[1], conv_v[0], conv_v[1]]).reshape(1, 4 * KV)
    filt_arr = np.ascontiguousarray(
        np.tile(filt_arr, (128, 1)), dtype=np.float32)
    rotm = _rot_matrix()

    in_maps = []
    for c in range(NCORES):
        b, s0 = c // 4, (c % 4) * CHUNK
        own = hidden[b, s0:s0 + CHUNK]
        prev = hidden[b, s0 - CHUNK:s0] if s0 > 0 else np.zeros_like(own)
        hid2 = np.concatenate([prev, own], axis=0)               # [1024,4096]
        hidT = np.ascontiguousarray(
            hid2.T.reshape(KT, 128, 1024).transpose(1, 0, 2)).astype(bf)
        costab, sintab, maskarr = _host_tables(positions[b], s0)
        in_maps.append({
            "hidT": hidT,
            "wpk": wpk,
            "wob": wob,
            "costab": costab,
            "sintab": sintab,
            "maskmul": np.ascontiguousarray(maskarr),
            "filt": filt_arr,
            "rotm": rotm,
        })
    return in_maps


def kernel(**inputs) -> np.ndarray:
    global _LAST_RESULTS
    from concourse.bass_utils import run_bass_kernel_spmd

    if REPEAT not in _PROGRAMS:
        _PROGRAMS[REPEAT] = _build_program(REPEAT)
    nc = _PROGRAMS[REPEAT]
    in_maps = _prepare_in_maps(inputs)

    kw = {}
    if TRACE:
        kw = dict(trace=True, trace_cores=[1], stitch_traces=False)
    res = run_bass_kernel_spmd(nc, in_maps, core_ids=list(range(NCORES)), **kw)
    _LAST_RESULTS = res

    out_full = np.empty((B, S, HID), dtype=np.float32)
    for c in range(NCORES):
        b, s0 = c // 4, (c % 4) * CHUNK
        out_full[b, s0:s0 + CHUNK] = res.results[c]["out"]
    return out_full


if __name__ == "__main__":
    rng = np.random.default_rng(0)
    ins = {
        "hidden_states": rng.standard_normal((B, S, HID)).astype(np.float32) * 0.02,
        "positions": np.broadcast_to(np.arange(S, dtype=np.int32), (B, S)).copy(),
        "w_pack": rng.standard_normal((HID, F)).astype(np.float32) * HID ** -0.5,
        "w_o": rng.standard_normal((H * D, HID)).astype(np.float32) * (H * D) ** -0.5,
        "conv_k": rng.standard_normal((2, KV)).astype(np.float32) * 0.5,
        "conv_v": rng.standard_normal((2, KV)).astype(np.float32) * 0.5,
    }
    out = kernel(**ins)
    print("kernel ran, out shape", out.shape, "finite:", np.isfinite(out).all())
